# revision 29
# baseline (speedup 1.0000x reference)
"""ArDCA loss on 8 TRN2 NeuronCores, data-parallel over M.

Per core (M/8 = 1024 sequences), TK=128 tiling (5376 = 42*128, no padding):
  - P^T = W^T @ H^T as causal fp8 DoubleRow matmuls on TensorE (W scaled x16,
    pre-transposed/masked/packed flat on host; one resident SBUF copy,
    per-tile strip DMAs); steady state runs at the 216ns/matmul fp8 roofline
  - Z path: Et = fp8(exp(P/16 + h)) on ScalarE; per-position partition sums
    accumulated in 2 persistent PSUM bank pairs via one-hot DoubleRow
    matmuls. Bank 1 (tiles 21..41) closes its partitions [0:96) early --
    they are final after pair u2=19 -- under the last two chains; only
    [96:128) remains in the tail. The last chain runs mc0 fully before mc1
    so Exp/Z/Ln of mc0 overlap the mc1 matmuls.
  - sel path + Ln path use DVE scalar_tensor_tensor with accum_out (fused
    multiply + free-axis reduce, one op each)
  - sel path reads the host-prepared w-scaled one-hot wh, streamed JIT on
    the gpsimd ring
  - startup: warm memset + dummy matmuls from ~8us ride out the p-state
    ramp; Exp activation table preloaded on ScalarE at t=0; hp DMAs split
    gpsimd(even)/sync(odd) so the Scalar queue carries only activations;
    oz one-hots in 3 chunks on sync; wb/o1 loaded mid-kernel
Host combines the two per-core scalars with exact-f32 h-gather and the
regularizer sums.
"""

import os
import sys

for _p in ("/opt/trn_rl_repo",):
    if _p not in sys.path:
        sys.path.append(_p)

from contextlib import ExitStack

import numpy as np
import ml_dtypes

import concourse.bass as bass
import concourse.mybir as mybir
from concourse import tile
from concourse.bass_utils import run_bass_kernel_spmd

BF16 = ml_dtypes.bfloat16
F8 = ml_dtypes.float8_e4m3
FP32 = mybir.dt.float32
BF = mybir.dt.bfloat16
FP8 = mybir.dt.float8e4

L, Q, M, NC = 256, 21, 8192, 8
LQ = L * Q  # 5376 = 42 * 128
TK, T, NU = 128, 42, 21
MS = M // NC  # 1024
NPR = [(t + 2) // 2 for t in range(T)]
PB = [sum(NPR[:t]) for t in range(T)]
NPAIRS = sum(NPR)  # 462
SCALE = 16.0
LAMBDA_H, LAMBDA_J = 1e-06, 1e-4
AF = mybir.ActivationFunctionType
OP = mybir.AluOpType
AX = mybir.AxisListType
DR = mybir.MatmulPerfMode.DoubleRow

# Z-matmul plan: Et pair u2 covers tiles (2u2, 2u2+1); positions 0..127 live
# in tiles 0..20 (bank 0), positions 128..255 in tiles 21..41 (bank 1).
# Pair u2=10 straddles -> two matmuls with one r-slot zeroed each.
# Entries: (u2, bank, zeroed_r or None)
ZPLAN = []
for _u2 in range(NU):
    _t0, _t1 = 2 * _u2, 2 * _u2 + 1
    _b0 = 0 if _t0 <= 20 else 1
    _b1 = 0 if _t1 <= 20 else 1
    if _b0 == _b1:
        ZPLAN.append((_u2, _b0, None))
    else:
        ZPLAN.append((_u2, 0, 1))
        ZPLAN.append((_u2, 1, 0))
NZ = len(ZPLAN)  # 22
ZTOT = {b: sum(1 for (_, bb, _) in ZPLAN if bb == b) for b in (0, 1)}
# Bank 1 keeps accumulating through u2=20, but its partitions [0:96) are
# final after u2=19 (tiles 40/41 only touch positions >= 224+19 -> columns
# >= 115): Ln+reduce them early, during the last two chains. Only columns
# [96:128) remain for the tail. Partition APs must be quad-aligned -> 96.
PSPLIT = 96
assert (128 * 40) // Q - 128 == 115


def _legalize_waits(nc):
    """Split >cap sync waits into preceding EventSemaphore instructions.

    This container's walrus accepts at most 1 wait per instruction (2 on
    EventSemaphore); Tile's final drain carries one wait per used processor.
    """
    n_split = 0
    for f in nc.m.functions:
        for bb in f.blocks:
            if not any(
                ins.sync_info
                and ins.sync_info.on_wait
                and len(ins.sync_info.on_wait)
                > (2 if isinstance(ins, mybir.InstEventSemaphore) else 1)
                for ins in bb.instructions
            ):
                continue
            new_list = []
            for ins in bb.instructions:
                si = ins.sync_info
                waits = list(si.on_wait) if si and si.on_wait else []
                cap = 2 if isinstance(ins, mybir.InstEventSemaphore) else 1
                if len(waits) > cap:
                    extra, keep = waits[:-cap], waits[-cap:]
                    for k in range(0, len(extra), 2):
                        ev = mybir.InstEventSemaphore(
                            name=f"EVSPLIT-{n_split}", ins=[], outs=[]
                        )
                        n_split += 1
                        ev.engine = ins.engine
                        ev.sync_info = mybir.SyncInfo(
                            on_wait=extra[k : k + 2], on_update=[]
                        )
                        new_list.append(ev)
                        nc.register_instruction(ev, overwrite=True)
                    si.on_wait = keep
                new_list.append(ins)
            try:
                bb.instructions = new_list
            except Exception:
                bb.instructions.clear()
                bb.instructions.extend(new_list)
    return n_split


def build_nc():
    nc = bass.Bass()
    wt_e = nc.declare_dram_parameter("wt", [TK, NPAIRS * 256], FP8, isOutput=False)
    hp_e = nc.declare_dram_parameter("hp", [NU, TK, 2, MS], FP8, isOutput=False)
    ht_e = nc.declare_dram_parameter("ht", [TK, T], FP32, isOutput=False)
    wh_e = nc.declare_dram_parameter("wh", [T, TK, MS], FP8, isOutput=False)
    wb_e = nc.declare_dram_parameter("wb", [TK, MS], FP32, isOutput=False)
    oz_e = nc.declare_dram_parameter("oz", [TK, NZ, 2, TK], FP8, isOutput=False)
    o1_e = nc.declare_dram_parameter("o1", [TK, 1], FP32, isOutput=False)
    out_e = nc.declare_dram_parameter("out", [1, 2], FP32, isOutput=True)

    NWARM = int(os.environ.get("KT_WARM", "8"))
    # dummy matmuls inserted before chain t: "t:count,t:count"
    BRD = dict(
        (int(a), int(b))
        for a, b in (
            kv.split(":")
            for kv in os.environ.get("KT_BRIDGE", "2:2,4:1").split(",")
            if kv
        )
    )
    TAILD = int(os.environ.get("KT_TAIL", "3"))
    ZDELAY = int(os.environ.get("KT_ZDELAY", "4"))
    HP_AHEAD = int(os.environ.get("KT_HPA", "4"))
    ST_AHEAD = int(os.environ.get("KT_STA", "6"))
    WH_AHEAD = int(os.environ.get("KT_WHA", "4"))
    WH_BUFS = int(os.environ.get("KT_WHB", "8"))
    WB_AT = int(os.environ.get("KT_WBAT", "10"))

    with tile.TileContext(nc) as tc, ExitStack() as ctx:
        cpool = ctx.enter_context(tc.tile_pool(name="const", bufs=1))
        htp = ctx.enter_context(tc.tile_pool(name="htp", bufs=1))
        whp = ctx.enter_context(tc.tile_pool(name="whp", bufs=1))
        ep = ctx.enter_context(tc.tile_pool(name="ep", bufs=1))
        accp = ctx.enter_context(tc.tile_pool(name="accp", bufs=1))
        pP = ctx.enter_context(tc.tile_pool(name="pP", bufs=2, space="PSUM"))
        pZ = ctx.enter_context(tc.tile_pool(name="pZ", bufs=1, space="PSUM"))

        # ---- warm-up: memset on vector (idle early; gpsimd starts hp0) ----
        warm = cpool.tile([TK, 512], FP8)
        nc.vector.memset(warm[:], 0.0)
        wlhsT = warm[:, 0:256].rearrange("p (r c) -> p r c", r=2)
        wrhs = warm[:].rearrange("p (r c) -> p r c", r=2)
        _di = [0]

        def dummy(n, tags=("za00", "za01", "za10", "za11")):
            for _ in range(n):
                zt = pZ.tile(
                    [TK, 512], FP32, tag=tags[_di[0] % len(tags)], name="zd"
                )
                nc.tensor.matmul(
                    zt[:, 0:256], wlhsT, wrhs, start=True, stop=True, perf_mode=DR
                )
                _di[0] += 1

        # ---- scalar ring: activation-table preload, then JIT hp DMAs ----
        scr1 = cpool.tile([TK, 1], FP32)
        scr2 = cpool.tile([TK, 1], FP32)
        nc.scalar.memzero(scr1[:])
        nc.scalar.activation(scr2[:], scr1[:], AF.Exp)

        dummy(NWARM)

        # htt (tiny, Exp bias) on scalar right after the table preload
        htt = cpool.tile([TK, T], FP32)
        nc.scalar.dma_start(htt[:], ht_e[:])

        hps = [
            htp.tile([TK, 2, MS], FP8, tag=f"hp{u}", name=f"hp{u}") for u in range(NU)
        ]
        hp_issued = set()

        def issue_hp(u, eng):
            if u < NU and u not in hp_issued:
                hp_issued.add(u)
                eng.dma_start(hps[u][:], hp_e[u])

        # hp pairs alternate rings: even u on gpsimd, odd u on sync, so the
        # scalar queue carries ONLY activations (Exps never queue behind
        # DMA-issue stalls)
        def hp_ring(u):
            return nc.gpsimd if u % 2 == 0 else nc.sync

        # w-scaled one-hot wh tiles (host fp8), JIT on gpsimd
        whs = {}

        def issue_wh(t):
            if t < T and t not in whs:
                wtile = whp.tile(
                    [TK, MS], FP8, tag="wh", bufs=WH_BUFS, name=f"wh{t}"
                )
                nc.gpsimd.dma_start(wtile[:], wh_e[t])
                whs[t] = wtile

        # oz one-hot slices in 3 chunks on the sync ring (tiny: 0.7MB
        # total); chunk k covers ZPLAN entries OZSPLIT[k]..OZSPLIT[k+1]
        OZSPLIT = (0, 4, 12, NZ)
        ozchunks = [
            cpool.tile([TK, OZSPLIT[k + 1] - OZSPLIT[k], 2, TK], FP8, name=f"ozc{k}")
            for k in range(3)
        ]

        def oz_ap(z):
            k = 0 if z < 4 else (1 if z < 12 else 2)
            return ozchunks[k][:, z - OZSPLIT[k]]

        def issue_ozchunk(k):
            nc.sync.dma_start(
                ozchunks[k][:], oz_e[:, OZSPLIT[k] : OZSPLIT[k + 1]]
            )

        issue_hp(0, nc.gpsimd)
        issue_wh(0)
        issue_wh(1)
        issue_hp(2, nc.gpsimd)
        issue_wh(2)
        issue_wh(3)
        for _t in range(4, WH_AHEAD):
            issue_wh(_t)

        # sync ring: strip0, hp odds, strips JIT
        strips = [
            cpool.tile([TK, NPR[t] * 256], FP8, name=f"wts{t}") for t in range(T)
        ]
        st_issued = set()

        def issue_strip(t):
            if t < T and t not in st_issued:
                st_issued.add(t)
                a, b = PB[t] * 256, (PB[t] + NPR[t]) * 256
                nc.sync.dma_start(strips[t][:], wt_e[:, a:b])

        issue_strip(0)
        issue_strip(1)
        issue_strip(2)
        issue_hp(1, nc.sync)
        issue_strip(3)
        issue_ozchunk(0)
        issue_hp(3, nc.sync)
        for _t in range(4, ST_AHEAD):
            issue_strip(_t)

        # wb (f32 weights, Ln path) + o1: loaded mid-kernel on gpsimd
        wb = cpool.tile([TK, MS], FP32)
        o1 = cpool.tile([TK, 1], FP32)

        # ---- persistent accumulators ----
        accSel = accp.tile([TK, 2 * T], FP32)
        accZw = accp.tile([TK, 6], FP32)
        zs2 = accp.tile([TK, 2], FP32)
        nc.vector.memset(accZw[:], 0.0)

        # Et pair buffers + Z accumulation banks
        ets = {}
        zacc = {}
        zemit = {0: 0, 1: 0}

        def close_slice(g, lo, hi, col0):
            # Ln + fused weighted reduce over partitions [lo:hi) of bank g
            for mc in range(2):
                lz = ep.tile([TK, 512], BF, tag="lz", bufs=2, name="lz")
                nc.scalar.activation(
                    lz[lo:hi, :], zacc[(g, mc)][lo:hi, :], AF.Ln
                )
                zc = ep.tile([TK, 512], BF, tag="zc", bufs=2, name="zc")
                nc.vector.scalar_tensor_tensor(
                    zc[lo:hi, :],
                    lz[lo:hi, :],
                    1.0,
                    wb[lo:hi, mc * 512 : (mc + 1) * 512],
                    op0=OP.mult,
                    op1=OP.mult,
                    accum_out=accZw[lo:hi, col0 + mc : col0 + mc + 1],
                )

        def emit_z(u2):
            etc = ets[u2]
            for z, (u2_, b, zr) in enumerate(ZPLAN):
                if u2_ != u2:
                    continue
                first = zemit[b] == 0
                zemit[b] += 1
                last = zemit[b] == ZTOT[b]
                for mc in range(2):
                    key = (b, mc)
                    if key not in zacc:
                        zacc[key] = pZ.tile(
                            [TK, 512], FP32, tag=f"za{b}{mc}", name=f"za{b}{mc}"
                        )
                    nc.tensor.matmul(
                        zacc[key][:],
                        oz_ap(z),
                        etc[mc][:],
                        start=first,
                        stop=last,
                        perf_mode=DR,
                    )
                if last and b == 0:
                    close_slice(0, 0, TK, 0)
                elif b == 1 and zemit[1] == ZTOT[1] - 1:
                    # partitions [0:96) of bank 1 are final after u2=19
                    # (tiles 40/41 only touch columns >= 115): close them
                    # while the PE runs the last two chains
                    close_slice(1, 0, PSPLIT, 2)
                elif last and b == 1:
                    close_slice(1, PSPLIT, TK, 4)

        # ---- main causal loop ----
        next_z = 0
        for t in range(T):
            if t == 2:
                issue_ozchunk(1)
            if t == 10:
                issue_ozchunk(2)
            issue_strip(t + ST_AHEAD)
            _u = t // 2 + HP_AHEAD
            issue_hp(_u, hp_ring(_u))
            issue_wh(t + WH_AHEAD)
            if t == WB_AT:
                nc.gpsimd.dma_start(wb[:], wb_e[:])
            if t == WB_AT + 2:
                nc.gpsimd.dma_start(o1[:], o1_e[:])
            if t in BRD:
                # route mid-loop dummies to still-closed g1 banks
                dummy(
                    BRD[t],
                    tags=(
                        ("za10", "za11")
                        if t >= 3
                        else ("za00", "za01", "za10", "za11")
                    ),
                )
            npr = NPR[t]
            Ps = [
                pP.tile([TK, 512], FP32, tag=f"P{mc}", name=f"P{mc}")
                for mc in range(2)
            ]
            # last chain: run all of mc0 before mc1, so Exp(41,mc0) and the
            # first tail Z-matmul overlap the mc1 half instead of serializing
            # after it
            mc_outer = t == T - 1
            for mc in range(2) if mc_outer else (0,):
                for u in range(npr):
                    a = u * 256
                    lhsT = strips[t][:, a : a + 256].rearrange(
                        "p (r c) -> p r c", r=2
                    )
                    for m in ((mc,) if mc_outer else (0, 1)):
                        nc.tensor.matmul(
                            Ps[m][:],
                            lhsT,
                            hps[u][:, :, m * 512 : (m + 1) * 512],
                            start=(u == 0),
                            stop=(u == npr - 1),
                            perf_mode=DR,
                        )
            # deferred Z matmuls: pair u2 complete at tile 2u2+1; give the
            # Exps some chains of slack before queueing behind this chain.
            # Late pairs get less slack so g1 closes during chain 40/41.
            while next_z < NU and 2 * next_z + 1 <= t - (
                ZDELAY if next_z < 17 else 1
            ):
                emit_z(next_z)
                next_z += 1
            # consumers of this chain
            u2c = t // 2
            if u2c not in ets:
                ets[u2c] = [
                    ep.tile([TK, 2, 512], FP8, tag=f"E{mc}", bufs=3, name=f"et{mc}")
                    for mc in range(2)
                ]
            for mc in range(2):
                nc.scalar.activation(
                    ets[u2c][mc][:, t % 2, :],
                    Ps[mc][:],
                    AF.Exp,
                    bias=htt[:, t : t + 1],
                    scale=1.0 / SCALE,
                )
            for mc in range(2):
                col = 2 * t + mc
                sc = ep.tile([TK, 512], BF, tag="sc", bufs=2, name="sc")
                nc.vector.scalar_tensor_tensor(
                    sc[:],
                    Ps[mc][:],
                    1.0,
                    whs[t][:, mc * 512 : (mc + 1) * 512],
                    op0=OP.mult,
                    op1=OP.mult,
                    accum_out=accSel[:, col : col + 1],
                )

        # ---- tail: sel total, flush g2, final reduce ----
        nc.vector.tensor_reduce(
            zs2[:, 1:2], accSel[:], axis=AX.X, op=OP.add
        )
        while next_z < NU:
            if next_z == NU - 1:
                # za0x only: the g1 banks' straddle partitions are still
                # read by the tail add, a dummy into them would corrupt
                dummy(TAILD, tags=("za00", "za01"))
            emit_z(next_z)
            next_z += 1

        nc.vector.tensor_reduce(
            zs2[:, 0:1], accZw[:], axis=AX.X, op=OP.add
        )
        pfin = pP.tile([1, 2], FP32, tag="P0", name="pfin")
        nc.tensor.matmul(pfin[:], o1[:], zs2[:], start=True, stop=True)
        ot = accp.tile([1, 2], FP32)
        nc.scalar.copy(ot[:], pfin[:])
        nc.sync.dma_start(out_e[:], ot[:])

    _legalize_waits(nc)
    return nc


_NC_CACHE = None
_CONST_CACHE = None


def _get_nc():
    global _NC_CACHE
    if _NC_CACHE is None:
        _NC_CACHE = build_nc()
    return _NC_CACHE


def _prep_consts():
    global _CONST_CACHE
    if _CONST_CACHE is None:
        oz = np.zeros((TK, NZ, 2, TK), dtype=F8)
        p = np.arange(TK)
        for z, (u2, g, zr) in enumerate(ZPLAN):
            for r in range(2):
                if zr == r:
                    continue
                pos = (256 * u2 + 128 * r + p) // Q
                c = pos - 128 * (1 if g >= 1 else 0)
                oz[p, z, r, c] = 1.0
        _CONST_CACHE = {
            "oz": oz,
            "o1": np.ones((TK, 1), np.float32),
        }
    return _CONST_CACHE


def _prep_inputs(seqs, weights, h, J):
    seqs = np.asarray(seqs)
    weights = np.ascontiguousarray(np.asarray(weights, dtype=np.float32))
    h = np.asarray(h, dtype=np.float32)
    J = np.asarray(J, dtype=np.float32)

    # W[jk, ia] = J[i, j, a, k], masked to pos(j) < pos(i), x16, fp8
    W = J.transpose(1, 3, 0, 2).reshape(LQ, LQ)
    pos = np.arange(LQ) // Q
    W8 = np.where(pos[:, None] < pos[None, :], W * SCALE, 0.0).astype(F8)
    W8v = W8.reshape(T, TK, T, TK)  # [jt, p, t, c]

    strips = []
    for t in range(T):
        blk = W8v[0 : 2 * NPR[t], :, t, :]  # [2npr, p, c]
        strips.append(
            blk.reshape(NPR[t], 2, TK, TK).transpose(2, 0, 1, 3).reshape(TK, -1)
        )
    wt = np.ascontiguousarray(np.concatenate(strips, axis=1))  # [TK, NPAIRS*256]

    # one-hot H^T (LQ, M)
    s32 = seqs.astype(np.int32)
    ohb = s32.T.repeat(Q, axis=0) == (np.arange(LQ, dtype=np.int32) % Q)[:, None]
    oh8 = ohb.astype(F8)
    hp = oh8.reshape(NU, 2, TK, M).transpose(0, 2, 1, 3)  # [NU, TK, 2, M]

    wh = (ohb * weights[None, :]).astype(F8).reshape(T, TK, M)

    ht = np.ascontiguousarray(h.reshape(T, TK).T)  # [TK, T]

    # exact host-side pieces (f64)
    hsel = h[np.arange(L)[None, :], s32].sum(axis=1).astype(np.float64)  # (M,)
    w64 = weights.astype(np.float64)
    hsel_w = float((hsel * w64).sum())
    wsum = float(w64.sum())
    j2 = (J.astype(np.float64) ** 2).sum(axis=(2, 3))
    sumW2 = float((j2 * np.tril(np.ones((L, L)), k=-1)).sum())
    sumh2 = float((h.astype(np.float64) ** 2).sum())

    consts = _prep_consts()
    in_maps = []
    for c in range(NC):
        sl = slice(c * MS, (c + 1) * MS)
        in_maps.append(
            {
                "wt": wt,
                "hp": np.ascontiguousarray(hp[..., sl]),
                "ht": ht,
                "wh": np.ascontiguousarray(wh[..., sl]),
                "wb": np.ascontiguousarray(
                    np.broadcast_to(weights[sl][None, :], (TK, MS))
                ),
                **consts,
            }
        )
    return in_maps, (hsel_w, wsum, sumW2, sumh2)


def _combine(results, hostsums):
    parts = np.stack([np.asarray(r["out"][0]) for r in results])  # (8, 2)
    Zw = float(parts[:, 0].sum())
    Uw = float(parts[:, 1].sum())
    hsel_w, wsum, sumW2, sumh2 = hostsums
    nll = (Zw - Uw / SCALE - hsel_w) / max(wsum, 1e-12)
    reg = 0.5 * LAMBDA_J * sumW2 + 0.5 * LAMBDA_H * sumh2
    loss = nll + reg
    return (
        np.float32(loss),
        np.float32(nll),
        np.float32(reg),
    )


def kernel(seqs, weights, h, J):
    nc = _get_nc()
    in_maps, hostsums = _prep_inputs(seqs, weights, h, J)
    res = run_bass_kernel_spmd(nc, in_maps, core_ids=list(range(NC)))
    return _combine(res.results, hostsums)


if __name__ == "__main__":
    d = np.load("/tmp/ref_data.npz")
    out = kernel(d["seqs"], d["weights"], d["h"], d["J"])
    print("kernel:", out)
    print("ref   :", d["loss"], d["nll"], d["reg"])


# revision 30
# speedup vs baseline: 1.0073x; 1.0073x over previous
"""ArDCA loss on 8 TRN2 NeuronCores, data-parallel over M.

Per core (M/8 = 1024 sequences), TK=128 tiling (5376 = 42*128, no padding):
  - P^T = W^T @ H^T as causal fp8 DoubleRow matmuls on TensorE (W scaled x16,
    pre-transposed/masked/packed flat on host; one resident SBUF copy,
    per-tile strip DMAs); steady state runs at the 216ns/matmul fp8 roofline
  - Z path: Et = fp8(exp(P/16 + h)) on ScalarE; per-position partition sums
    accumulated in 2 persistent PSUM bank pairs via one-hot DoubleRow
    matmuls. Bank 1 (tiles 21..41) closes its partitions [0:96) early --
    they are final after pair u2=19 -- under the last two chains; only
    [96:128) remains in the tail. The last chain runs mc0 fully before mc1
    so Exp/Z/Ln of mc0 overlap the mc1 matmuls.
  - sel path + Ln path use DVE scalar_tensor_tensor with accum_out (fused
    multiply + free-axis reduce, one op each)
  - sel path reads the host-prepared w-scaled one-hot wh, streamed JIT on
    the gpsimd ring
  - startup: warm memset + dummy matmuls from ~8us ride out the p-state
    ramp; Exp activation table preloaded on ScalarE at t=0; hp DMAs split
    gpsimd(even)/sync(odd) so the Scalar queue carries only activations;
    oz one-hots in 3 chunks on sync; wb/o1 loaded mid-kernel
Host combines the two per-core scalars with exact-f32 h-gather and the
regularizer sums.
"""

import os
import sys

for _p in ("/opt/trn_rl_repo",):
    if _p not in sys.path:
        sys.path.append(_p)

from contextlib import ExitStack

import numpy as np
import ml_dtypes

import concourse.bass as bass
import concourse.mybir as mybir
from concourse import tile
from concourse.bass_utils import run_bass_kernel_spmd

BF16 = ml_dtypes.bfloat16
F8 = ml_dtypes.float8_e4m3
FP32 = mybir.dt.float32
BF = mybir.dt.bfloat16
FP8 = mybir.dt.float8e4

L, Q, M, NC = 256, 21, 8192, 8
LQ = L * Q  # 5376 = 42 * 128
TK, T, NU = 128, 42, 21
MS = M // NC  # 1024
NPR = [(t + 2) // 2 for t in range(T)]
PB = [sum(NPR[:t]) for t in range(T)]
NPAIRS = sum(NPR)  # 462
SCALE = 16.0
LAMBDA_H, LAMBDA_J = 1e-06, 1e-4
AF = mybir.ActivationFunctionType
OP = mybir.AluOpType
AX = mybir.AxisListType
DR = mybir.MatmulPerfMode.DoubleRow

# Z-matmul plan: Et pair u2 covers tiles (2u2, 2u2+1); positions 0..127 live
# in tiles 0..20 (bank 0), positions 128..255 in tiles 21..41 (bank 1).
# Pair u2=10 straddles -> two matmuls with one r-slot zeroed each.
# Entries: (u2, bank, zeroed_r or None)
ZPLAN = []
for _u2 in range(NU):
    _t0, _t1 = 2 * _u2, 2 * _u2 + 1
    _b0 = 0 if _t0 <= 20 else 1
    _b1 = 0 if _t1 <= 20 else 1
    if _b0 == _b1:
        ZPLAN.append((_u2, _b0, None))
    else:
        ZPLAN.append((_u2, 0, 1))
        ZPLAN.append((_u2, 1, 0))
NZ = len(ZPLAN)  # 22
ZTOT = {b: sum(1 for (_, bb, _) in ZPLAN if bb == b) for b in (0, 1)}
# Bank 1 keeps accumulating through u2=20, but its partitions [0:96) are
# final after u2=19 (tiles 40/41 only touch positions >= 224+19 -> columns
# >= 115): Ln+reduce them early, during the last two chains. Only columns
# [96:128) remain for the tail. Partition APs must be quad-aligned -> 96.
PSPLIT = 96
assert (128 * 40) // Q - 128 == 115


def _legalize_waits(nc):
    """Split >cap sync waits into preceding EventSemaphore instructions.

    This container's walrus accepts at most 1 wait per instruction (2 on
    EventSemaphore); Tile's final drain carries one wait per used processor.
    """
    n_split = 0
    for f in nc.m.functions:
        for bb in f.blocks:
            if not any(
                ins.sync_info
                and ins.sync_info.on_wait
                and len(ins.sync_info.on_wait)
                > (2 if isinstance(ins, mybir.InstEventSemaphore) else 1)
                for ins in bb.instructions
            ):
                continue
            new_list = []
            for ins in bb.instructions:
                si = ins.sync_info
                waits = list(si.on_wait) if si and si.on_wait else []
                cap = 2 if isinstance(ins, mybir.InstEventSemaphore) else 1
                if len(waits) > cap:
                    extra, keep = waits[:-cap], waits[-cap:]
                    for k in range(0, len(extra), 2):
                        ev = mybir.InstEventSemaphore(
                            name=f"EVSPLIT-{n_split}", ins=[], outs=[]
                        )
                        n_split += 1
                        ev.engine = ins.engine
                        ev.sync_info = mybir.SyncInfo(
                            on_wait=extra[k : k + 2], on_update=[]
                        )
                        new_list.append(ev)
                        nc.register_instruction(ev, overwrite=True)
                    si.on_wait = keep
                new_list.append(ins)
            try:
                bb.instructions = new_list
            except Exception:
                bb.instructions.clear()
                bb.instructions.extend(new_list)
    return n_split


def build_nc():
    nc = bass.Bass()
    wt_e = nc.declare_dram_parameter("wt", [TK, NPAIRS * 256], FP8, isOutput=False)
    hp_e = nc.declare_dram_parameter("hp", [NU, TK, 2, MS], FP8, isOutput=False)
    ht_e = nc.declare_dram_parameter("ht", [TK, T], FP32, isOutput=False)
    wh_e = nc.declare_dram_parameter("wh", [T, TK, MS], FP8, isOutput=False)
    wb_e = nc.declare_dram_parameter("wb", [TK, MS], FP32, isOutput=False)
    oz_e = nc.declare_dram_parameter("oz", [TK, NZ, 2, TK], FP8, isOutput=False)
    o1_e = nc.declare_dram_parameter("o1", [TK, 1], FP32, isOutput=False)
    out_e = nc.declare_dram_parameter("out", [1, 2], FP32, isOutput=True)

    NWARM = int(os.environ.get("KT_WARM", "8"))
    # dummy matmuls inserted before chain t: "t:count,t:count"
    BRD = dict(
        (int(a), int(b))
        for a, b in (
            kv.split(":")
            for kv in os.environ.get("KT_BRIDGE", "2:2,4:1").split(",")
            if kv
        )
    )
    TAILD = int(os.environ.get("KT_TAIL", "3"))
    ZDELAY = int(os.environ.get("KT_ZDELAY", "4"))
    HP_AHEAD = int(os.environ.get("KT_HPA", "4"))
    ST_AHEAD = int(os.environ.get("KT_STA", "6"))
    WH_AHEAD = int(os.environ.get("KT_WHA", "4"))
    WH_BUFS = int(os.environ.get("KT_WHB", "8"))
    WB_AT = int(os.environ.get("KT_WBAT", "10"))

    with tile.TileContext(nc) as tc, ExitStack() as ctx:
        cpool = ctx.enter_context(tc.tile_pool(name="const", bufs=1))
        htp = ctx.enter_context(tc.tile_pool(name="htp", bufs=1))
        whp = ctx.enter_context(tc.tile_pool(name="whp", bufs=1))
        ep = ctx.enter_context(tc.tile_pool(name="ep", bufs=1))
        accp = ctx.enter_context(tc.tile_pool(name="accp", bufs=1))
        pP = ctx.enter_context(tc.tile_pool(name="pP", bufs=2, space="PSUM"))
        pZ = ctx.enter_context(tc.tile_pool(name="pZ", bufs=1, space="PSUM"))

        # ---- warm-up: memset on vector (idle early; gpsimd starts hp0) ----
        warm = cpool.tile([TK, 512], FP8)
        nc.vector.memset(warm[:], 0.0)
        wlhsT = warm[:, 0:256].rearrange("p (r c) -> p r c", r=2)
        wrhs = warm[:].rearrange("p (r c) -> p r c", r=2)
        _di = [0]

        def dummy(n, tags=("za00", "za01", "za10", "za11")):
            for _ in range(n):
                zt = pZ.tile(
                    [TK, 512], FP32, tag=tags[_di[0] % len(tags)], name="zd"
                )
                nc.tensor.matmul(
                    zt[:, 0:256], wlhsT, wrhs, start=True, stop=True, perf_mode=DR
                )
                _di[0] += 1

        # ---- scalar ring: hp0's high half, activation-table preload ----
        scr1 = cpool.tile([TK, 1], FP32)
        scr2 = cpool.tile([TK, 1], FP32)
        nc.scalar.memzero(scr1[:])
        nc.scalar.activation(scr2[:], scr1[:], AF.Exp)

        dummy(NWARM)

        # htt (tiny, Exp bias) on scalar right after the table preload
        htt = cpool.tile([TK, T], FP32)
        nc.scalar.dma_start(htt[:], ht_e[:])

        hps = [
            htp.tile([TK, 2, MS], FP8, tag=f"hp{u}", name=f"hp{u}") for u in range(NU)
        ]
        hp_issued = set()

        def issue_hp(u, eng):
            if u < NU and u not in hp_issued:
                hp_issued.add(u)
                eng.dma_start(hps[u][:], hp_e[u])

        # hp pairs alternate rings: even u on gpsimd, odd u on sync, so the
        # scalar queue carries ONLY activations (Exps never queue behind
        # DMA-issue stalls)
        def hp_ring(u):
            return nc.gpsimd if u % 2 == 0 else nc.sync

        # w-scaled one-hot wh tiles (host fp8), JIT on gpsimd
        whs = {}

        def issue_wh(t):
            if t < T and t not in whs:
                wtile = whp.tile(
                    [TK, MS], FP8, tag="wh", bufs=WH_BUFS, name=f"wh{t}"
                )
                nc.gpsimd.dma_start(wtile[:], wh_e[t])
                whs[t] = wtile

        # oz one-hot slices in 3 chunks on the sync ring (tiny: 0.7MB
        # total); chunk k covers ZPLAN entries OZSPLIT[k]..OZSPLIT[k+1]
        OZSPLIT = (0, 4, 12, NZ)
        ozchunks = [
            cpool.tile([TK, OZSPLIT[k + 1] - OZSPLIT[k], 2, TK], FP8, name=f"ozc{k}")
            for k in range(3)
        ]

        def oz_ap(z):
            k = 0 if z < 4 else (1 if z < 12 else 2)
            return ozchunks[k][:, z - OZSPLIT[k]]

        def issue_ozchunk(k):
            nc.sync.dma_start(
                ozchunks[k][:], oz_e[:, OZSPLIT[k] : OZSPLIT[k + 1]]
            )

        # hp0 split into column halves on two queues so chain 0 starts ~1us
        # sooner (low half gpsimd, high half scalar ahead of the table load)
        hp_issued.add(0)
        nc.gpsimd.dma_start(hps[0][:, :, 0:512], hp_e[0, :, :, 0:512])
        nc.scalar.dma_start(hps[0][:, :, 512:MS], hp_e[0, :, :, 512:MS])
        issue_wh(0)
        issue_wh(1)
        issue_hp(2, nc.gpsimd)
        issue_wh(2)
        issue_wh(3)
        for _t in range(4, WH_AHEAD):
            issue_wh(_t)

        # sync ring: strip0, hp odds, strips JIT
        strips = [
            cpool.tile([TK, NPR[t] * 256], FP8, name=f"wts{t}") for t in range(T)
        ]
        st_issued = set()

        def issue_strip(t):
            if t < T and t not in st_issued:
                st_issued.add(t)
                a, b = PB[t] * 256, (PB[t] + NPR[t]) * 256
                nc.sync.dma_start(strips[t][:], wt_e[:, a:b])

        issue_strip(0)
        issue_strip(1)
        issue_hp(1, nc.sync)
        issue_strip(2)
        issue_ozchunk(0)
        issue_strip(3)
        issue_hp(3, nc.sync)
        for _t in range(4, ST_AHEAD):
            issue_strip(_t)

        # wb (f32 weights, Ln path) + o1: loaded mid-kernel on gpsimd
        wb = cpool.tile([TK, MS], FP32)
        o1 = cpool.tile([TK, 1], FP32)

        # ---- persistent accumulators ----
        accSel = accp.tile([TK, 2 * T], FP32)
        accZw = accp.tile([TK, 6], FP32)
        zs2 = accp.tile([TK, 2], FP32)
        nc.vector.memset(accZw[:], 0.0)

        # Et pair buffers + Z accumulation banks
        ets = {}
        zacc = {}
        zemit = {0: 0, 1: 0}

        def close_slice(g, lo, hi, col0):
            # Ln + fused weighted reduce over partitions [lo:hi) of bank g
            for mc in range(2):
                lz = ep.tile([TK, 512], BF, tag="lz", bufs=2, name="lz")
                nc.scalar.activation(
                    lz[lo:hi, :], zacc[(g, mc)][lo:hi, :], AF.Ln
                )
                zc = ep.tile([TK, 512], BF, tag="zc", bufs=2, name="zc")
                nc.vector.scalar_tensor_tensor(
                    zc[lo:hi, :],
                    lz[lo:hi, :],
                    1.0,
                    wb[lo:hi, mc * 512 : (mc + 1) * 512],
                    op0=OP.mult,
                    op1=OP.mult,
                    accum_out=accZw[lo:hi, col0 + mc : col0 + mc + 1],
                )

        def emit_z(u2):
            etc = ets[u2]
            for z, (u2_, b, zr) in enumerate(ZPLAN):
                if u2_ != u2:
                    continue
                first = zemit[b] == 0
                zemit[b] += 1
                last = zemit[b] == ZTOT[b]
                for mc in range(2):
                    key = (b, mc)
                    if key not in zacc:
                        zacc[key] = pZ.tile(
                            [TK, 512], FP32, tag=f"za{b}{mc}", name=f"za{b}{mc}"
                        )
                    nc.tensor.matmul(
                        zacc[key][:],
                        oz_ap(z),
                        etc[mc][:],
                        start=first,
                        stop=last,
                        perf_mode=DR,
                    )
                if last and b == 0:
                    close_slice(0, 0, TK, 0)
                elif b == 1 and zemit[1] == ZTOT[1] - 1:
                    # partitions [0:96) of bank 1 are final after u2=19
                    # (tiles 40/41 only touch columns >= 115): close them
                    # while the PE runs the last two chains
                    close_slice(1, 0, PSPLIT, 2)
                elif last and b == 1:
                    close_slice(1, PSPLIT, TK, 4)

        # ---- main causal loop ----
        next_z = 0
        for t in range(T):
            if t == 2:
                issue_ozchunk(1)
            if t == 10:
                issue_ozchunk(2)
            issue_strip(t + ST_AHEAD)
            _u = t // 2 + HP_AHEAD
            issue_hp(_u, hp_ring(_u))
            issue_wh(t + WH_AHEAD)
            if t == WB_AT:
                nc.gpsimd.dma_start(wb[:], wb_e[:])
            if t == WB_AT + 2:
                nc.gpsimd.dma_start(o1[:], o1_e[:])
            if t in BRD:
                # route mid-loop dummies to still-closed g1 banks
                dummy(
                    BRD[t],
                    tags=(
                        ("za10", "za11")
                        if t >= 3
                        else ("za00", "za01", "za10", "za11")
                    ),
                )
            npr = NPR[t]
            Ps = [
                pP.tile([TK, 512], FP32, tag=f"P{mc}", name=f"P{mc}")
                for mc in range(2)
            ]
            # last chain: run all of mc0 before mc1, so Exp(41,mc0) and the
            # first tail Z-matmul overlap the mc1 half instead of serializing
            # after it
            mc_outer = t == T - 1
            for mc in range(2) if mc_outer else (0,):
                for u in range(npr):
                    a = u * 256
                    lhsT = strips[t][:, a : a + 256].rearrange(
                        "p (r c) -> p r c", r=2
                    )
                    for m in ((mc,) if mc_outer else (0, 1)):
                        nc.tensor.matmul(
                            Ps[m][:],
                            lhsT,
                            hps[u][:, :, m * 512 : (m + 1) * 512],
                            start=(u == 0),
                            stop=(u == npr - 1),
                            perf_mode=DR,
                        )
            # deferred Z matmuls: pair u2 complete at tile 2u2+1; give the
            # Exps some chains of slack before queueing behind this chain.
            # Late pairs get less slack so g1 closes during chain 40/41.
            while next_z < NU and 2 * next_z + 1 <= t - (
                ZDELAY if next_z < 17 else 1
            ):
                emit_z(next_z)
                next_z += 1
            # consumers of this chain
            u2c = t // 2
            if u2c not in ets:
                ets[u2c] = [
                    ep.tile([TK, 2, 512], FP8, tag=f"E{mc}", bufs=3, name=f"et{mc}")
                    for mc in range(2)
                ]
            for mc in range(2):
                nc.scalar.activation(
                    ets[u2c][mc][:, t % 2, :],
                    Ps[mc][:],
                    AF.Exp,
                    bias=htt[:, t : t + 1],
                    scale=1.0 / SCALE,
                )
            for mc in range(2):
                col = 2 * t + mc
                sc = ep.tile([TK, 512], BF, tag="sc", bufs=2, name="sc")
                nc.vector.scalar_tensor_tensor(
                    sc[:],
                    Ps[mc][:],
                    1.0,
                    whs[t][:, mc * 512 : (mc + 1) * 512],
                    op0=OP.mult,
                    op1=OP.mult,
                    accum_out=accSel[:, col : col + 1],
                )

        # ---- tail: sel total, flush g2, final reduce ----
        nc.vector.tensor_reduce(
            zs2[:, 1:2], accSel[:], axis=AX.X, op=OP.add
        )
        while next_z < NU:
            if next_z == NU - 1:
                # za0x only: the g1 banks' straddle partitions are still
                # read by the tail add, a dummy into them would corrupt
                dummy(TAILD, tags=("za00", "za01"))
            emit_z(next_z)
            next_z += 1

        nc.vector.tensor_reduce(
            zs2[:, 0:1], accZw[:], axis=AX.X, op=OP.add
        )
        pfin = pP.tile([1, 2], FP32, tag="P0", name="pfin")
        nc.tensor.matmul(pfin[:], o1[:], zs2[:], start=True, stop=True)
        ot = accp.tile([1, 2], FP32)
        nc.scalar.copy(ot[:], pfin[:])
        nc.sync.dma_start(out_e[:], ot[:])

    _legalize_waits(nc)
    return nc


_NC_CACHE = None
_CONST_CACHE = None


def _get_nc():
    global _NC_CACHE
    if _NC_CACHE is None:
        _NC_CACHE = build_nc()
    return _NC_CACHE


def _prep_consts():
    global _CONST_CACHE
    if _CONST_CACHE is None:
        oz = np.zeros((TK, NZ, 2, TK), dtype=F8)
        p = np.arange(TK)
        for z, (u2, g, zr) in enumerate(ZPLAN):
            for r in range(2):
                if zr == r:
                    continue
                pos = (256 * u2 + 128 * r + p) // Q
                c = pos - 128 * (1 if g >= 1 else 0)
                oz[p, z, r, c] = 1.0
        _CONST_CACHE = {
            "oz": oz,
            "o1": np.ones((TK, 1), np.float32),
        }
    return _CONST_CACHE


def _prep_inputs(seqs, weights, h, J):
    seqs = np.asarray(seqs)
    weights = np.ascontiguousarray(np.asarray(weights, dtype=np.float32))
    h = np.asarray(h, dtype=np.float32)
    J = np.asarray(J, dtype=np.float32)

    # W[jk, ia] = J[i, j, a, k], masked to pos(j) < pos(i), x16, fp8
    W = J.transpose(1, 3, 0, 2).reshape(LQ, LQ)
    pos = np.arange(LQ) // Q
    W8 = np.where(pos[:, None] < pos[None, :], W * SCALE, 0.0).astype(F8)
    W8v = W8.reshape(T, TK, T, TK)  # [jt, p, t, c]

    strips = []
    for t in range(T):
        blk = W8v[0 : 2 * NPR[t], :, t, :]  # [2npr, p, c]
        strips.append(
            blk.reshape(NPR[t], 2, TK, TK).transpose(2, 0, 1, 3).reshape(TK, -1)
        )
    wt = np.ascontiguousarray(np.concatenate(strips, axis=1))  # [TK, NPAIRS*256]

    # one-hot H^T (LQ, M)
    s32 = seqs.astype(np.int32)
    ohb = s32.T.repeat(Q, axis=0) == (np.arange(LQ, dtype=np.int32) % Q)[:, None]
    oh8 = ohb.astype(F8)
    hp = oh8.reshape(NU, 2, TK, M).transpose(0, 2, 1, 3)  # [NU, TK, 2, M]

    wh = (ohb * weights[None, :]).astype(F8).reshape(T, TK, M)

    ht = np.ascontiguousarray(h.reshape(T, TK).T)  # [TK, T]

    # exact host-side pieces (f64)
    hsel = h[np.arange(L)[None, :], s32].sum(axis=1).astype(np.float64)  # (M,)
    w64 = weights.astype(np.float64)
    hsel_w = float((hsel * w64).sum())
    wsum = float(w64.sum())
    j2 = (J.astype(np.float64) ** 2).sum(axis=(2, 3))
    sumW2 = float((j2 * np.tril(np.ones((L, L)), k=-1)).sum())
    sumh2 = float((h.astype(np.float64) ** 2).sum())

    consts = _prep_consts()
    in_maps = []
    for c in range(NC):
        sl = slice(c * MS, (c + 1) * MS)
        in_maps.append(
            {
                "wt": wt,
                "hp": np.ascontiguousarray(hp[..., sl]),
                "ht": ht,
                "wh": np.ascontiguousarray(wh[..., sl]),
                "wb": np.ascontiguousarray(
                    np.broadcast_to(weights[sl][None, :], (TK, MS))
                ),
                **consts,
            }
        )
    return in_maps, (hsel_w, wsum, sumW2, sumh2)


def _combine(results, hostsums):
    parts = np.stack([np.asarray(r["out"][0]) for r in results])  # (8, 2)
    Zw = float(parts[:, 0].sum())
    Uw = float(parts[:, 1].sum())
    hsel_w, wsum, sumW2, sumh2 = hostsums
    nll = (Zw - Uw / SCALE - hsel_w) / max(wsum, 1e-12)
    reg = 0.5 * LAMBDA_J * sumW2 + 0.5 * LAMBDA_H * sumh2
    loss = nll + reg
    return (
        np.float32(loss),
        np.float32(nll),
        np.float32(reg),
    )


def kernel(seqs, weights, h, J):
    nc = _get_nc()
    in_maps, hostsums = _prep_inputs(seqs, weights, h, J)
    res = run_bass_kernel_spmd(nc, in_maps, core_ids=list(range(NC)))
    return _combine(res.results, hostsums)


if __name__ == "__main__":
    d = np.load("/tmp/ref_data.npz")
    out = kernel(d["seqs"], d["weights"], d["h"], d["J"])
    print("kernel:", out)
    print("ref   :", d["loss"], d["nll"], d["reg"])
